# revision 1
# baseline (speedup 1.0000x reference)
"""vq_codebook kernel for trn2: cosine-sim argmax over K=65536 codes + codebook gather.

Strategy: shard K across 8 cores. Per core (slab Kc=8192):
  - fp16 matmul screen: sims = targ @ (W * diag(1/colnorm))  (row norms don't
    affect the argmax over k; eps is absorbed by the host-side margin check)
  - PE -> PSUM fp32; ACT copies PSUM -> SBUF fp16; DVE computes, per 128-row
    block, an elementwise max over the 8 interleaved planes sims[:, j*8+c]
    (c = k mod 8) in 3 tensor_max ops, then max8/max_index over the 1024-wide
    root -> top position j* and top-2 root values.
  - candidates k in [8*j*, 8*j*+8) are contiguous: one indirect DMA per block
    gathers the 8 candidate codebook rows (fp32, exact) from the W^T slab.
Host: exactly rescores the 8 candidates per core (the gathered rows ARE the
codebook vectors) in float64, picks the global winner among 64 candidates,
and fully recomputes any row where a screened-out code could beat the best
candidate (second root value + error band >= best candidate sim).
"""

import os
import sys

import numpy as np

for _p in ("/opt/trn_rl_repo", "/root/.axon_site/_ro/trn_rl_repo"):
    if os.path.isdir(_p) and _p not in sys.path:
        sys.path.append(_p)

import concourse.bass as bass
import concourse.bass_isa as bass_isa
import concourse.tile as tile
from concourse import bacc, mybir
from concourse.bass import IndirectOffsetOnAxis
from concourse.bass_utils import run_bass_kernel_spmd

P = 128
B, D, K, NCORES = 8192, 256, 65536, 8
KC = K // NCORES  # 8192 per-core codebook slab
NCH = 8           # interleave factor: candidate group = k mod NCH
EPS = 1e-7

# cosine-unit bound on |fp16 screen - exact| incl. fp16 sims quantization
# (measured 2.6e-4 worst-case on seed-0; 3x safety)
BAND = 8.0e-4

F32 = mybir.dt.float32
F16 = mybir.dt.float16
U32 = mybir.dt.uint32
AF = mybir.ActivationFunctionType
ALU = mybir.AluOpType


def build_core_kernel(nc, b=B, d=D, kc=KC, qw=2048, pck=512):
    """Emit the per-core kernel. b: batch rows, d: feature dim (must be 256),
    kc: per-core codebook columns, qw: PSUM quarter width, pck: prologue
    chunk width."""
    assert d == 2 * P
    mb = b // P           # number of 128-row blocks
    nq = kc // qw         # PSUM quarters per block
    nj = kc // NCH        # root width (candidate-group count)

    tT = nc.dram_tensor("tT", [d, b], F32, kind="ExternalInput")
    w = nc.dram_tensor("w", [d, kc], F32, kind="ExternalInput")
    wT = nc.dram_tensor("wT", [kc, d], F32, kind="ExternalInput")
    g1_d = nc.dram_tensor("g1", [P, mb], F32, kind="ExternalOutput")
    g2_d = nc.dram_tensor("g2", [P, mb], F32, kind="ExternalOutput")
    jpos_d = nc.dram_tensor("jpos", [P, mb], U32, kind="ExternalOutput")
    rows_d = nc.dram_tensor("rows8", [b, NCH * d], F32, kind="ExternalOutput")
    invb = nc.dram_tensor("invb", [1, kc], F32)  # internal bounce for 1/colnorm

    with tile.TileContext(nc) as tc:
        with (
            tc.tile_pool(name="persist", bufs=1) as persist,
            tc.tile_pool(name="stage", bufs=max(2, 2048 // pck)) as stage,
            tc.tile_pool(name="sq", bufs=2 if pck <= 512 else 1) as sqp,
            tc.tile_pool(name="cn", bufs=1) as cnp,
            tc.tile_pool(name="sims", bufs=3) as simsp,
            tc.tile_pool(name="tree", bufs=1) as treep,
            tc.tile_pool(name="small", bufs=4) as smallp,
            tc.tile_pool(name="rowout", bufs=3) as rowp,
            tc.tile_pool(name="psum", bufs=2, space="PSUM") as psump,
        ):
            # ---- persistent tiles ----
            Tn = persist.tile([P, 2 * b], F16)    # targ^T, fp16
            Wn = persist.tile([P, 2 * kc], F16)   # col-normalized W, fp16
            g1w = persist.tile([P, mb], F32)
            g2w = persist.tile([P, mb], F32)
            jw = persist.tile([P, mb], U32)

            # ---- prologue: load targ^T and W as fp16 via SWDGE cast-DMA ----
            # (W is read from HBM exactly once; no fp32 staging tiles at all)
            ldk = min(2048, kc)
            for c in range(kc // ldk):
                sl = slice(c * ldk, (c + 1) * ldk)
                nc.gpsimd.dma_start(out=Wn[:, c * ldk : (c + 1) * ldk], in_=w[0:P, sl])
                nc.gpsimd.dma_start(
                    out=Wn[:, kc + c * ldk : kc + (c + 1) * ldk], in_=w[P : 2 * P, sl]
                )
            ldb = min(4096, b)
            for c in range(b // ldb):
                sl = slice(c * ldb, (c + 1) * ldb)
                nc.gpsimd.dma_start(out=Tn[:, c * ldb : (c + 1) * ldb], in_=tT[0:P, sl])
                nc.gpsimd.dma_start(
                    out=Tn[:, b + c * ldb : b + (c + 1) * ldb], in_=tT[P : 2 * P, sl]
                )

            # column norms from the fp16 Wn (error ~3e-5 relative, absorbed
            # by the host-side margin band), processed in two halves so the
            # first half of Wn is normalized (and matmuls can start) while the
            # second half is still loading.
            nck = kc // pck
            jwid2 = (kc // 2) // P
            for h in range(2):
                hc0 = h * (nck // 2)
                for c in range(hc0, hc0 + nck // 2):
                    sl = slice(c * pck, (c + 1) * pck)
                    sqa = sqp.tile([P, pck], F32, tag="sqa")
                    sqb = sqp.tile([P, pck], F32, tag="sqb")
                    nc.scalar.activation(
                        sqa[:], Wn[:, c * pck : (c + 1) * pck], AF.Square
                    )
                    nc.scalar.activation(
                        sqb[:], Wn[:, kc + c * pck : kc + (c + 1) * pck], AF.Square
                    )
                    wss = sqp.tile([P, pck], F32, tag="wss")
                    nc.vector.tensor_add(wss[:], sqa[:], sqb[:])
                    pr = sqp.tile([P, pck], F32, tag="pr")
                    nc.gpsimd.partition_all_reduce(
                        pr[:], wss[:], channels=P, reduce_op=bass_isa.ReduceOp.add
                    )
                    nc.sync.dma_start(out=invb[0:1, sl], in_=pr[0:1, :])

                hsl = slice(h * (kc // 2), (h + 1) * (kc // 2))
                cn2 = cnp.tile([P, jwid2], F32, tag="cn2")
                nc.sync.dma_start(
                    out=cn2[:],
                    in_=invb[0:1, hsl].rearrange("o (p j) -> (o p) j", p=P),
                )
                srt = cnp.tile([P, jwid2], F32, tag="srt")
                nc.scalar.activation(srt[:], cn2[:], AF.Sqrt)
                u0 = cnp.tile([P, jwid2], F32, tag="u0")
                nc.vector.reciprocal(u0[:], srt[:])
                uu = cnp.tile([P, jwid2], F32, tag="uu")
                nc.vector.tensor_mul(uu[:], u0[:], u0[:])
                nc.vector.tensor_mul(uu[:], uu[:], cn2[:])
                nc.vector.tensor_scalar(
                    uu[:], uu[:], -0.5, 1.5, op0=ALU.mult, op1=ALU.add
                )
                u1 = cnp.tile([P, jwid2], F32, tag="u1")
                nc.vector.tensor_mul(u1[:], u0[:], uu[:])
                nc.sync.dma_start(
                    out=invb[0:1, hsl].rearrange("o (p j) -> (o p) j", p=P),
                    in_=u1[:],
                )

                # scale this half of Wn in place
                for c in range(hc0, hc0 + nck // 2):
                    sl = slice(c * pck, (c + 1) * pck)
                    icb = stage.tile([P, pck], F32, tag="icb")
                    nc.sync.dma_start(
                        out=icb[:], in_=invb[0:1, sl].to_broadcast([P, pck])
                    )
                    nc.vector.tensor_mul(
                        Wn[:, c * pck : (c + 1) * pck],
                        Wn[:, c * pck : (c + 1) * pck],
                        icb[:],
                    )
                    nc.vector.tensor_mul(
                        Wn[:, kc + c * pck : kc + (c + 1) * pck],
                        Wn[:, kc + c * pck : kc + (c + 1) * pck],
                        icb[:],
                    )

            # view of the W^T slab as candidate groups of NCH consecutive rows
            wT_g = wT[:].rearrange("(a e) d -> a (e d)", e=NCH)

            # ---- main loop over 128-row blocks ----
            for m in range(mb):
                S = simsp.tile([P, kc], F16)
                for q in range(nq):
                    pq = psump.tile([P, qw], F32, space="PSUM")
                    for i in range(2):
                        lhsT = Tn[:, i * b + m * P : i * b + (m + 1) * P]
                        for cc in range(qw // 512):
                            k0 = q * qw + cc * 512
                            nc.tensor.matmul(
                                out=pq[:, cc * 512 : (cc + 1) * 512],
                                lhsT=lhsT,
                                rhs=Wn[:, i * kc + k0 : i * kc + k0 + 512],
                                start=(i == 0),
                                stop=(i == 1),
                            )
                    nc.scalar.activation(
                        S[:, q * qw : (q + 1) * qw], pq[:], AF.Copy, bias=0.0
                    )

                # elementwise max over the NCH=8 interleaved planes (c = k%8)
                S3 = S[:].rearrange("p (j c) -> p j c", c=NCH)
                t1 = treep.tile([P, nj * 4], F16, tag="t1")
                t1v = t1[:].rearrange("p (j c) -> p j c", c=4)
                nc.vector.tensor_max(t1v[:, :, :], S3[:, :, 0:4], S3[:, :, 4:8])
                t2 = treep.tile([P, nj * 2], F16, tag="t2")
                t2v = t2[:].rearrange("p (j c) -> p j c", c=2)
                nc.vector.tensor_max(t2v[:, :, :], t1v[:, :, 0:2], t1v[:, :, 2:4])
                root = treep.tile([P, nj], F16, tag="root")
                nc.vector.tensor_max(root[:], t2v[:, :, 0], t2v[:, :, 1])

                r8 = smallp.tile([P, 8], F16, tag="r8")
                nc.vector.max(out=r8[:], in_=root[:])
                j8 = smallp.tile([P, 8], U32, tag="j8")
                nc.vector.max_index(out=j8[:], in_max=r8[:], in_values=root[:])
                nc.vector.tensor_copy(jw[:, m : m + 1], j8[:, 0:1])
                nc.vector.tensor_copy(g1w[:, m : m + 1], r8[:, 0:1])
                nc.vector.tensor_copy(g2w[:, m : m + 1], r8[:, 1:2])

                rowt = rowp.tile([P, NCH * d], F32)
                nc.gpsimd.indirect_dma_start(
                    out=rowt[:],
                    out_offset=None,
                    in_=wT_g,
                    in_offset=IndirectOffsetOnAxis(ap=jw[:, m : m + 1], axis=0),
                )
                nc.sync.dma_start(out=rows_d[m * P : (m + 1) * P, :], in_=rowt[:])

            nc.sync.dma_start(out=g1_d[:], in_=g1w[:])
            nc.sync.dma_start(out=g2_d[:], in_=g2w[:])
            nc.sync.dma_start(out=jpos_d[:], in_=jw[:])

    nc.compile()
    return nc


_CACHE = {}
LAST_RESULT = None
LAST_AMB = -1


def _get_nc():
    if "nc" not in _CACHE:
        nc = bacc.Bacc(
            "TRN2", target_bir_lowering=False, debug=False, enable_asserts=False
        )
        build_core_kernel(nc)
        _CACHE["nc"] = nc
    return _CACHE["nc"]


def _unpack_vec(arr):
    # [128, mb] with b = m*128 + p  ->  [b]
    return np.ascontiguousarray(arr.T).ravel()


def kernel(targ: np.ndarray, W: np.ndarray) -> np.ndarray:
    assert targ.shape == (B, D) and W.shape == (D, K)
    targ = np.ascontiguousarray(targ, dtype=np.float32)
    W = np.ascontiguousarray(W, dtype=np.float32)
    nc = _get_nc()

    tT = np.ascontiguousarray(targ.T)
    in_maps = []
    for c in range(NCORES):
        wslab = np.ascontiguousarray(W[:, c * KC : (c + 1) * KC])
        in_maps.append({"tT": tT, "w": wslab, "wT": np.ascontiguousarray(wslab.T)})

    global LAST_RESULT
    LAST_RESULT = run_bass_kernel_spmd(nc, in_maps, list(range(NCORES)))
    res = LAST_RESULT.results

    g2 = np.stack([_unpack_vec(r["g2"]) for r in res])            # [NC, B]
    jpos = np.stack([_unpack_vec(r["jpos"]) for r in res])        # [NC, B]
    rows8 = np.stack([r["rows8"].reshape(B, NCH, D) for r in res])  # [NC,B,8,D]

    # exact rescore of the NCORES*NCH candidates per row (float64)
    t64 = targ.astype(np.float64)
    rown = np.linalg.norm(t64, axis=1)
    cand = rows8.transpose(1, 0, 2, 3).reshape(B, NCORES * NCH, D)  # k-ordered
    c64 = cand.astype(np.float64)
    dots = np.einsum("bkd,bd->bk", c64, t64)
    cnorm = np.linalg.norm(c64, axis=2)
    sims = dots / (rown[:, None] * cnorm + EPS)
    best_c = np.argmax(sims, axis=1)                 # first max = smallest k
    best_cos = sims[np.arange(B), best_c]
    out = cand[np.arange(B), best_c, :].astype(np.float32)

    # any non-candidate code k on core c has screen value <= g2[c,b], hence
    # exact cosine <= g2[c,b]/||t_b|| + BAND.  Accept iff best candidate beats
    # that bound.
    bound = g2.max(axis=0) / rown + BAND
    # also guard candidate-vs-candidate near-ties (fp32 reference could order
    # them differently than our f64 rescore)
    s_sorted = np.sort(sims, axis=1)
    cand_tie = (s_sorted[:, -1] - s_sorted[:, -2]) < 1e-6
    amb = np.where((best_cos < bound) | cand_tie)[0]
    global LAST_AMB
    LAST_AMB = len(amb)
    if len(amb):
        col_nm = np.linalg.norm(W, axis=0)
        t_amb = targ[amb]
        s = (t_amb @ W) / (
            np.linalg.norm(targ[amb], axis=1)[:, None] * col_nm[None, :] + EPS
        )
        k_star = np.argmax(s, axis=1)
        out[amb] = W[:, k_star].T
    return out



# revision 3
# speedup vs baseline: 1.0051x; 1.0051x over previous
"""vq_codebook trn2 kernel v2: fp8e4 DoubleRow screen + pair-max funnel.

Per core (K sharded 8 ways, slab Kc=8192):
  - host pre-normalizes W columns / t rows, scales by 16, quantizes to fp8e4
  - PE: DoubleRow fp8 matmuls (full d=256 contraction per op) -> PSUM fp32
    screen values = 256 * (cos + quantization err), |err| <= BAND
  - funnel per 2048-col PSUM quarter: roots = max over adjacent code PAIRS
    (C=2). For most pair-groups ACT copies plane-1 to SBUF fp16 and one DVE
    tensor_max (single PSUM operand - HW rule) drains plane-0 and writes the
    root. For a tunable tail of groups ACT copies both planes (plane-outer
    layout) and DVE does a cheap fp16 2x self-max. No tree, no on-device
    argmax.
  - roots [128, 4096] fp16 per row-block DMA'd to HBM
Host: top-T pair-groups per row over the 32768 roots, exact rescore of the
2T candidate codes (f64), bound check vs (T+1)-th root + BAND; flagged rows
get a full fp32 row rescore mirroring the reference.
"""

import os
import sys

import numpy as np
import ml_dtypes

for _p in ("/opt/trn_rl_repo", "/root/.axon_site/_ro/trn_rl_repo"):
    if os.path.isdir(_p) and _p not in sys.path:
        sys.path.append(_p)

import concourse.bass as bass
import concourse.tile as tile
from concourse import bacc, mybir
from concourse.bass_utils import run_bass_kernel_spmd

P = 128
B, D, K, NCORES = 8192, 256, 65536, 8
KC = K // NCORES          # 8192 codes per core
C = 2                     # codes per root group (adjacent pair)
QW = 2048                 # PSUM quarter width
GQ = QW // C              # pair-groups per quarter (1024)
NG = KC // C              # 4096 roots per core row
SCALE = 16.0
EPS = 1e-7
BAND = 0.0155             # |screen/256 - cos| bound (measured 0.0133 + margin)
ULP16 = 0.1               # fp16 ulp at screen magnitude ~90 (raw units)
T_GROUPS = 64             # pair-groups rescored exactly per row

F32 = mybir.dt.float32
F16 = mybir.dt.float16
F8 = mybir.dt.float8e4
AF = mybir.ActivationFunctionType
DR = mybir.MatmulPerfMode.DoubleRow

# Codes j and j+SQ (adjacent 1024-wide sub-quarters) form a pair-group:
# ACT copies even sub-quarters to SBUF fp16; DVE tensor_max pairs the odd
# sub-quarter's PSUM against the copy (one PSUM operand - HW rule) writing
# the root segment directly. The first GS columns of one odd sub-quarter
# are ACT-copied instead and self-maxed on DVE at fp16 2x to balance
# ACT/DVE load.
SQ = 1024                 # sub-quarter width
GS = 352                  # self-pair slice width (on sub-quarter 5)


def build_core_kernel(nc, mb=B // P, gs=GS, gs_map=None):
    # gs_map: optional {odd sub-quarter index -> self-slice width}
    if gs_map is None:
        gs_map = {5: gs}
    b = mb * P
    nsq = KC // SQ            # 8 sub-quarters

    t8 = nc.dram_tensor("t8", [2 * P, b], F8, kind="ExternalInput")
    w8 = nc.dram_tensor("w8", [2 * P, KC], F8, kind="ExternalInput")
    root_d = nc.dram_tensor("roots", [b, NG], F16, kind="ExternalOutput")

    with tile.TileContext(nc) as tc:
        with (
            tc.tile_pool(name="persist", bufs=1) as persist,
            tc.tile_pool(name="s", bufs=3) as sp,
            tc.tile_pool(name="s2", bufs=3) as s2p,
            tc.tile_pool(name="root", bufs=3) as rootp,
            tc.tile_pool(name="psum", bufs=4, space="PSUM") as psump,
        ):
            Tn = persist.tile([P, 2 * b], F8)
            Wn = persist.tile([P, 2 * KC], F8)
            # first T chunk (block 0 needs it) via HWDGE; W via gpsimd SWDGE
            # (separate descriptor generator) interleaved by half so early
            # matmuls unblock ASAP (subtile deps)
            ldb = min(512, b)
            for h in range(2):
                nc.sync.dma_start(
                    out=Tn[:, h * b : h * b + ldb], in_=t8[h * P : (h + 1) * P, 0:ldb]
                )
            ldk = 1024
            for c0 in range(0, KC, ldk):
                for h in range(2):
                    nc.sync.dma_start(
                        out=Wn[:, h * KC + c0 : h * KC + c0 + ldk],
                        in_=w8[h * P : (h + 1) * P, c0 : c0 + ldk],
                    )
            for c0 in range(ldb, b, 2048):
                ld = min(2048, b - c0)
                for h in range(2):
                    nc.sync.dma_start(
                        out=Tn[:, h * b + c0 : h * b + c0 + ld],
                        in_=t8[h * P : (h + 1) * P, c0 : c0 + ld],
                    )
            Tv = Tn[:].rearrange("p (two b) -> p two b", two=2)
            Wv = Wn[:].rearrange("p (two n) -> p two n", two=2)

            for m in range(mb):
                lhsT = Tv[:, :, m * P : (m + 1) * P]
                root = rootp.tile([P, NG], F16)
                Seven = None
                for sq in range(nsq):
                    pq = psump.tile([P, SQ], F32, space="PSUM")
                    for i in range(SQ // 512):
                        k0 = sq * SQ + i * 512
                        nc.tensor.matmul(
                            out=pq[:, i * 512 : (i + 1) * 512],
                            lhsT=lhsT,
                            rhs=Wv[:, :, k0 : k0 + 512],
                            start=True,
                            stop=True,
                            perf_mode=DR,
                        )
                    h = sq // 2
                    rseg = root[:, h * SQ : (h + 1) * SQ]
                    if sq % 2 == 0:
                        # ACT: copy the whole even sub-quarter
                        Seven = sp.tile([P, SQ], F16)
                        nc.scalar.activation(Seven[:], pq[:], AF.Copy, bias=0.0)
                    else:
                        g0 = gs_map.get(sq, 0)
                        if g0:
                            # ACT: copy the self-slice of the odd sub-quarter
                            Sodd = s2p.tile([P, g0], F16)
                            nc.scalar.activation(
                                Sodd[:], pq[:, 0:g0], AF.Copy, bias=0.0
                            )
                            # DVE fp16 2x self-max
                            nc.vector.tensor_max(
                                rseg[:, 0:g0], Sodd[:], Seven[:, 0:g0]
                            )
                        # DVE: root = max(odd PSUM, even copy)
                        nc.vector.tensor_max(
                            rseg[:, g0:SQ], pq[:, g0:SQ], Seven[:, g0:SQ]
                        )
                nc.sync.dma_start(out=root_d[m * P : (m + 1) * P, :], in_=root[:])

    nc.compile()
    return nc


_CACHE = {}
LAST_RESULT = None
LAST_AMB = -1


def _get_nc():
    if "nc" not in _CACHE:
        nc = bacc.Bacc(
            "TRN2", target_bir_lowering=False, debug=False, enable_asserts=False
        )
        build_core_kernel(nc)
        _CACHE["nc"] = nc
    return _CACHE["nc"]


def kernel(targ: np.ndarray, W: np.ndarray) -> np.ndarray:
    assert targ.shape == (B, D) and W.shape == (D, K)
    targ = np.ascontiguousarray(targ, dtype=np.float32)
    W = np.ascontiguousarray(W, dtype=np.float32)
    nc = _get_nc()

    # host prep: normalize, scale, fp8e4-quantize
    rown = np.linalg.norm(targ, axis=1)
    t8 = np.ascontiguousarray(
        (SCALE * (targ / rown[:, None])).T.astype(ml_dtypes.float8_e4m3)
    )  # [256, B]
    coln = np.linalg.norm(W, axis=0)
    W8 = (SCALE * (W / coln[None, :])).astype(ml_dtypes.float8_e4m3)  # [256, K]

    in_maps = []
    for c in range(NCORES):
        in_maps.append(
            {"t8": t8, "w8": np.ascontiguousarray(W8[:, c * KC : (c + 1) * KC])}
        )

    global LAST_RESULT, LAST_AMB
    LAST_RESULT = run_bass_kernel_spmd(nc, in_maps, list(range(NCORES)))
    res = LAST_RESULT.results

    # roots: [NCORES, B, NG] -> [B, NCORES*NG]; pair gid = core*NG + local
    roots = np.concatenate(
        [r["roots"].astype(np.float32) for r in res], axis=1
    )  # [B, 32768]

    # top-T pair groups + (T+1)-th root for the bound
    part = np.argpartition(-roots, T_GROUPS, axis=1)
    topT = part[:, :T_GROUPS]  # [B, T]
    rT1 = np.take_along_axis(roots, part[:, T_GROUPS : T_GROUPS + 1], axis=1)[:, 0]

    # decode pair gid -> 2 candidate code columns: local gid j = h*SQ + n
    # pairs codes {h*2*SQ + n, h*2*SQ + SQ + n} of that core's slab
    core_id = topT // NG
    within = topT % NG
    h = within // SQ
    n = within % SQ
    k1 = core_id * KC + h * 2 * SQ + n  # [B, T]
    cands = np.stack([k1, k1 + SQ], axis=2).reshape(B, T_GROUPS * C)

    # exact rescore (f64) in row chunks
    Wt = np.ascontiguousarray(W.T)  # [K, D]
    best_cos = np.empty(B, dtype=np.float64)
    second_cos = np.empty(B, dtype=np.float64)
    best_k = np.empty(B, dtype=np.int64)
    CH = 1024
    for r0 in range(0, B, CH):
        r1 = min(B, r0 + CH)
        vec = Wt[cands[r0:r1]].astype(np.float64)  # [ch, 2T, D]
        tch = targ[r0:r1].astype(np.float64)
        dots = np.einsum("rcd,rd->rc", vec, tch)
        cn = np.linalg.norm(vec, axis=2)
        cos = dots / (rown[r0:r1, None].astype(np.float64) * cn + EPS)
        o = np.argsort(-cos, axis=1)
        n = r1 - r0
        best = o[:, 0]
        best_cos[r0:r1] = cos[np.arange(n), best]
        second_cos[r0:r1] = cos[np.arange(n), o[:, 1]]
        best_k[r0:r1] = cands[r0:r1][np.arange(n), best]

    out = Wt[best_k].astype(np.float32)  # [B, D]

    # bound: any code outside the top-T pairs has screen <= rT1 (+ulp),
    # so exact cos <= (rT1+ulp)/256 + BAND. Flag rows where the best
    # candidate doesn't clearly beat that, plus near-ties among candidates.
    bound = (rT1.astype(np.float64) + ULP16) / (SCALE * SCALE) + BAND
    amb = np.where((best_cos < bound) | (best_cos - second_cos < 1e-6))[0]
    LAST_AMB = len(amb)
    if len(amb):
        # full fp32 mirror of the reference for flagged rows
        s = (targ[amb] @ W) / (rown[amb][:, None] * coln[None, :] + EPS)
        k_star = np.argmax(s, axis=1)
        out[amb] = W[:, k_star].T
    return out


if __name__ == "__main__":
    # micro-test: mb blocks, 1 core, random data -> roots vs numpy + sim time
    import time

    from concourse.timeline_sim import TimelineSim

    nc2 = None
    for mb in (2, 6):
        nc = bacc.Bacc(
            "TRN2", target_bir_lowering=False, debug=False, enable_asserts=False
        )
        build_core_kernel(nc, mb=mb)
        t0 = time.time()
        dur = TimelineSim(nc, trace=False).simulate()
        print(
            f"TimelineSim mb={mb}: {dur:.0f} ns (per-block est "
            f"{(dur - 29583 if mb == 6 else 0)/4 if mb == 6 else 0:.0f})",
            flush=True,
        )
        if mb == 2:
            nc2 = nc

    rng = np.random.default_rng(1)
    mb = 2
    t8 = (rng.standard_normal((256, mb * P))).astype(ml_dtypes.float8_e4m3)
    w8 = (rng.standard_normal((256, KC)) * 0.5).astype(ml_dtypes.float8_e4m3)
    res = run_bass_kernel_spmd(nc2, [{"t8": t8, "w8": w8}], [0])
    roots = res.results[0]["roots"].astype(np.float32)  # [mb*P, 4096]
    screen = t8.astype(np.float32).T @ w8.astype(np.float32)
    ref_roots = (
        screen.reshape(mb * P, 4, 2, SQ)
        .max(axis=2)
        .reshape(mb * P, NG)
        .astype(np.float16)
        .astype(np.float32)
    )
    err = np.abs(roots - ref_roots)
    print(
        f"roots err: max {err.max():.5f} rel {err.max()/np.abs(ref_roots).max():.2e}",
        flush=True,
    )


# revision 6
# speedup vs baseline: 1.0140x; 1.0089x over previous
"""vq_codebook trn2 kernel v2: fp8e4 DoubleRow screen + pair-max funnel.

Per core (K sharded 8 ways, slab Kc=8192):
  - host pre-normalizes W columns / t rows, scales by 16, quantizes to fp8e4
  - PE: DoubleRow fp8 matmuls (full d=256 contraction per op) -> PSUM fp32
    screen values = 256 * (cos + quantization err), |err| <= BAND
  - funnel: codes j and j+1024 (adjacent 1024-wide PSUM sub-quarters) form a
    pair-group. ACT copies even sub-quarters to SBUF fp16; one DVE tensor_max
    (single PSUM operand - HW rule) drains the odd sub-quarter and writes the
    pair-max root directly. A tunable slice (GS cols of one odd sub-quarter)
    is ACT-copied too and self-maxed on DVE at fp16 2x to balance ACT vs DVE
    load. No tree, no on-device argmax.
  - roots [128, 4096] fp16 per row-block DMA'd to HBM
Host: top-T pair-groups per row over the 32768 roots, exact rescore of the
2T candidate codes (f64), bound check vs (T+1)-th root + BAND; flagged rows
get a full fp32 row rescore mirroring the reference.
"""

import os
import sys

import numpy as np
import ml_dtypes

for _p in ("/opt/trn_rl_repo", "/root/.axon_site/_ro/trn_rl_repo"):
    if os.path.isdir(_p) and _p not in sys.path:
        sys.path.append(_p)

import concourse.bass as bass
import concourse.tile as tile
from concourse import bacc, mybir
from concourse.bass_utils import run_bass_kernel_spmd

P = 128
B, D, K, NCORES = 8192, 256, 65536, 8
KC = K // NCORES          # 8192 codes per core
C = 2                     # codes per root group (adjacent pair)
QW = 2048                 # PSUM quarter width
GQ = QW // C              # pair-groups per quarter (1024)
NG = KC // C              # 4096 roots per core row
SCALE = 16.0
EPS = 1e-7
BAND = 0.0155             # |screen/256 - cos| bound (measured 0.0133 + margin)
ULP16 = 0.1               # fp16 ulp at screen magnitude ~90 (raw units)
T_GROUPS = 64             # pair-groups rescored exactly per row

F32 = mybir.dt.float32
F16 = mybir.dt.float16
F8 = mybir.dt.float8e4
AF = mybir.ActivationFunctionType
DR = mybir.MatmulPerfMode.DoubleRow

# Codes j and j+SQ (adjacent 1024-wide sub-quarters) form a pair-group:
# ACT copies even sub-quarters to SBUF fp16; DVE tensor_max pairs the odd
# sub-quarter's PSUM against the copy (one PSUM operand - HW rule) writing
# the root segment directly. The first GS columns of one odd sub-quarter
# are ACT-copied instead and self-maxed on DVE at fp16 2x to balance
# ACT/DVE load.
SQ = 1024                 # sub-quarter width
GS = 352                  # self-pair slice width (on sub-quarter 5)


def build_core_kernel(nc, mb=B // P, gs=GS, gs_map=None):
    # gs_map: optional {odd sub-quarter index -> self-slice width}
    if gs_map is None:
        gs_map = {5: gs}
    b = mb * P
    nsq = KC // SQ            # 8 sub-quarters

    t8 = nc.dram_tensor("t8", [2 * P, b], F8, kind="ExternalInput")
    w8 = nc.dram_tensor("w8", [2 * P, KC], F8, kind="ExternalInput")
    root_d = nc.dram_tensor("roots", [b, NG], F16, kind="ExternalOutput")

    with tile.TileContext(nc) as tc:
        with (
            tc.tile_pool(name="persist", bufs=1) as persist,
            tc.tile_pool(name="s", bufs=3) as sp,
            tc.tile_pool(name="s2", bufs=3) as s2p,
            tc.tile_pool(name="root", bufs=3) as rootp,
            tc.tile_pool(name="psum", bufs=4, space="PSUM") as psump,
        ):
            Tn = persist.tile([P, 2 * b], F8)
            Wn = persist.tile([P, 2 * KC], F8)
            # first T chunk (block 0 needs it) via HWDGE; W via gpsimd SWDGE
            # (separate descriptor generator) interleaved by half so early
            # matmuls unblock ASAP (subtile deps)
            ldb = min(512, b)
            for h in range(2):
                nc.sync.dma_start(
                    out=Tn[:, h * b : h * b + ldb], in_=t8[h * P : (h + 1) * P, 0:ldb]
                )
            ldk = 1024
            for c0 in range(0, KC, ldk):
                for h in range(2):
                    # halves go through separate descriptor generators
                    # (HWDGE via sync, SWDGE via gpsimd) to overlap dge time
                    eng_dma = nc.sync if h == 0 else nc.gpsimd
                    eng_dma.dma_start(
                        out=Wn[:, h * KC + c0 : h * KC + c0 + ldk],
                        in_=w8[h * P : (h + 1) * P, c0 : c0 + ldk],
                    )
            for c0 in range(ldb, b, 2048):
                ld = min(2048, b - c0)
                for h in range(2):
                    eng_dma = nc.sync if h == 0 else nc.gpsimd
                    eng_dma.dma_start(
                        out=Tn[:, h * b + c0 : h * b + c0 + ld],
                        in_=t8[h * P : (h + 1) * P, c0 : c0 + ld],
                    )
            Tv = Tn[:].rearrange("p (two b) -> p two b", two=2)
            Wv = Wn[:].rearrange("p (two n) -> p two n", two=2)

            for m in range(mb):
                lhsT = Tv[:, :, m * P : (m + 1) * P]
                root = rootp.tile([P, NG], F16)
                Seven = None
                for sq in range(nsq):
                    pq = psump.tile([P, SQ], F32, space="PSUM")
                    for i in range(SQ // 512):
                        k0 = sq * SQ + i * 512
                        nc.tensor.matmul(
                            out=pq[:, i * 512 : (i + 1) * 512],
                            lhsT=lhsT,
                            rhs=Wv[:, :, k0 : k0 + 512],
                            start=True,
                            stop=True,
                            perf_mode=DR,
                        )
                    h = sq // 2
                    rseg = root[:, h * SQ : (h + 1) * SQ]
                    if sq % 2 == 0:
                        # ACT: copy the whole even sub-quarter
                        Seven = sp.tile([P, SQ], F16)
                        nc.scalar.activation(Seven[:], pq[:], AF.Copy, bias=0.0)
                    else:
                        g0 = gs_map.get(sq, 0)
                        if g0:
                            # ACT: copy the self-slice of the odd sub-quarter
                            Sodd = s2p.tile([P, g0], F16)
                            nc.scalar.activation(
                                Sodd[:], pq[:, 0:g0], AF.Copy, bias=0.0
                            )
                            # DVE fp16 2x self-max
                            nc.vector.tensor_max(
                                rseg[:, 0:g0], Sodd[:], Seven[:, 0:g0]
                            )
                        # DVE: root = max(odd PSUM, even copy)
                        nc.vector.tensor_max(
                            rseg[:, g0:SQ], pq[:, g0:SQ], Seven[:, g0:SQ]
                        )
                nc.sync.dma_start(out=root_d[m * P : (m + 1) * P, :], in_=root[:])

    nc.compile()
    return nc


_CACHE = {}
LAST_RESULT = None
LAST_AMB = -1


def _get_nc():
    if "nc" not in _CACHE:
        nc = bacc.Bacc(
            "TRN2", target_bir_lowering=False, debug=False, enable_asserts=False
        )
        build_core_kernel(nc)
        _CACHE["nc"] = nc
    return _CACHE["nc"]


def kernel(targ: np.ndarray, W: np.ndarray) -> np.ndarray:
    assert targ.shape == (B, D) and W.shape == (D, K)
    targ = np.ascontiguousarray(targ, dtype=np.float32)
    W = np.ascontiguousarray(W, dtype=np.float32)
    nc = _get_nc()

    # host prep: normalize, scale, fp8e4-quantize
    rown = np.linalg.norm(targ, axis=1)
    t8 = np.ascontiguousarray(
        (SCALE * (targ / rown[:, None])).T.astype(ml_dtypes.float8_e4m3)
    )  # [256, B]
    coln = np.linalg.norm(W, axis=0)
    W8 = (SCALE * (W / coln[None, :])).astype(ml_dtypes.float8_e4m3)  # [256, K]

    in_maps = []
    for c in range(NCORES):
        in_maps.append(
            {"t8": t8, "w8": np.ascontiguousarray(W8[:, c * KC : (c + 1) * KC])}
        )

    global LAST_RESULT, LAST_AMB
    LAST_RESULT = run_bass_kernel_spmd(nc, in_maps, list(range(NCORES)))
    res = LAST_RESULT.results

    # roots: [NCORES, B, NG] -> [B, NCORES*NG]; pair gid = core*NG + local
    roots = np.concatenate(
        [r["roots"].astype(np.float32) for r in res], axis=1
    )  # [B, 32768]

    # top-T pair groups + (T+1)-th root for the bound
    part = np.argpartition(-roots, T_GROUPS, axis=1)
    topT = part[:, :T_GROUPS]  # [B, T]
    rT1 = np.take_along_axis(roots, part[:, T_GROUPS : T_GROUPS + 1], axis=1)[:, 0]

    # decode pair gid -> 2 candidate code columns: local gid j = h*SQ + n
    # pairs codes {h*2*SQ + n, h*2*SQ + SQ + n} of that core's slab
    core_id = topT // NG
    within = topT % NG
    h = within // SQ
    n = within % SQ
    k1 = core_id * KC + h * 2 * SQ + n  # [B, T]
    cands = np.stack([k1, k1 + SQ], axis=2).reshape(B, T_GROUPS * C)

    # exact rescore (f64) in row chunks
    Wt = np.ascontiguousarray(W.T)  # [K, D]
    best_cos = np.empty(B, dtype=np.float64)
    second_cos = np.empty(B, dtype=np.float64)
    best_k = np.empty(B, dtype=np.int64)
    CH = 1024
    for r0 in range(0, B, CH):
        r1 = min(B, r0 + CH)
        vec = Wt[cands[r0:r1]].astype(np.float64)  # [ch, 2T, D]
        tch = targ[r0:r1].astype(np.float64)
        dots = np.einsum("rcd,rd->rc", vec, tch)
        cn = np.linalg.norm(vec, axis=2)
        cos = dots / (rown[r0:r1, None].astype(np.float64) * cn + EPS)
        o = np.argsort(-cos, axis=1)
        nr = r1 - r0
        best = o[:, 0]
        best_cos[r0:r1] = cos[np.arange(nr), best]
        second_cos[r0:r1] = cos[np.arange(nr), o[:, 1]]
        best_k[r0:r1] = cands[r0:r1][np.arange(nr), best]

    out = Wt[best_k].astype(np.float32)  # [B, D]

    # bound: any code outside the top-T pairs has screen <= rT1 (+ulp),
    # so exact cos <= (rT1+ulp)/256 + BAND. Flag rows where the best
    # candidate doesn't clearly beat that, plus near-ties among candidates.
    bound = (rT1.astype(np.float64) + ULP16) / (SCALE * SCALE) + BAND
    amb = np.where((best_cos < bound) | (best_cos - second_cos < 1e-6))[0]
    LAST_AMB = len(amb)
    if len(amb):
        # full fp32 mirror of the reference for flagged rows
        s = (targ[amb] @ W) / (rown[amb][:, None] * coln[None, :] + EPS)
        k_star = np.argmax(s, axis=1)
        out[amb] = W[:, k_star].T
    return out


if __name__ == "__main__":
    # micro-test: mb blocks, 1 core, random data -> roots vs numpy + sim time
    import time

    from concourse.timeline_sim import TimelineSim

    nc2 = None
    for mb in (2, 6):
        nc = bacc.Bacc(
            "TRN2", target_bir_lowering=False, debug=False, enable_asserts=False
        )
        build_core_kernel(nc, mb=mb)
        t0 = time.time()
        dur = TimelineSim(nc, trace=False).simulate()
        print(
            f"TimelineSim mb={mb}: {dur:.0f} ns (per-block est "
            f"{(dur - 29583 if mb == 6 else 0)/4 if mb == 6 else 0:.0f})",
            flush=True,
        )
        if mb == 2:
            nc2 = nc

    rng = np.random.default_rng(1)
    mb = 2
    t8 = (rng.standard_normal((256, mb * P))).astype(ml_dtypes.float8_e4m3)
    w8 = (rng.standard_normal((256, KC)) * 0.5).astype(ml_dtypes.float8_e4m3)
    res = run_bass_kernel_spmd(nc2, [{"t8": t8, "w8": w8}], [0])
    roots = res.results[0]["roots"].astype(np.float32)  # [mb*P, 4096]
    screen = t8.astype(np.float32).T @ w8.astype(np.float32)
    ref_roots = (
        screen.reshape(mb * P, 4, 2, SQ)
        .max(axis=2)
        .reshape(mb * P, NG)
        .astype(np.float16)
        .astype(np.float32)
    )
    err = np.abs(roots - ref_roots)
    print(
        f"roots err: max {err.max():.5f} rel {err.max()/np.abs(ref_roots).max():.2e}",
        flush=True,
    )


# revision 7
# speedup vs baseline: 1.0212x; 1.0070x over previous
"""vq_codebook trn2 kernel v2: fp8e4 DoubleRow screen + pair-max funnel.

Per core (K sharded 8 ways, slab Kc=8192):
  - host pre-normalizes W columns / t rows, scales by 16, quantizes to fp8e4
  - PE: DoubleRow fp8 matmuls (full d=256 contraction per op) -> PSUM fp32
    screen values = 256 * (cos + quantization err), |err| <= BAND
  - funnel: codes j and j+1024 (adjacent 1024-wide PSUM sub-quarters) form a
    pair-group. ACT copies even sub-quarters to SBUF fp16; one DVE tensor_max
    (single PSUM operand - HW rule) drains the odd sub-quarter and writes the
    pair-max root directly. A tunable slice (GS cols of one odd sub-quarter)
    is ACT-copied too and self-maxed on DVE at fp16 2x to balance ACT vs DVE
    load. No tree, no on-device argmax.
  - roots [128, 4096] fp16 per row-block DMA'd to HBM
Host: top-T pair-groups per row over the 32768 roots, exact rescore of the
2T candidate codes (f64), bound check vs (T+1)-th root + BAND; flagged rows
get a full fp32 row rescore mirroring the reference.
"""

import os
import sys

import numpy as np
import ml_dtypes

for _p in ("/opt/trn_rl_repo", "/root/.axon_site/_ro/trn_rl_repo"):
    if os.path.isdir(_p) and _p not in sys.path:
        sys.path.append(_p)

import concourse.bass as bass
import concourse.tile as tile
from concourse import bacc, mybir
from concourse.bass_utils import run_bass_kernel_spmd

P = 128
B, D, K, NCORES = 8192, 256, 65536, 8
KC = K // NCORES          # 8192 codes per core
C = 2                     # codes per root group (adjacent pair)
QW = 2048                 # PSUM quarter width
GQ = QW // C              # pair-groups per quarter (1024)
NG = KC // C              # 4096 roots per core row
SCALE = 16.0
EPS = 1e-7
BAND = 0.0155             # |screen/256 - cos| bound (measured 0.0133 + margin)
ULP16 = 0.1               # fp16 ulp at screen magnitude ~90 (raw units)
T_GROUPS = 64             # pair-groups rescored exactly per row

F32 = mybir.dt.float32
F16 = mybir.dt.float16
F8 = mybir.dt.float8e4
AF = mybir.ActivationFunctionType
DR = mybir.MatmulPerfMode.DoubleRow

# Codes j and j+SQ (adjacent 1024-wide sub-quarters) form a pair-group:
# ACT copies even sub-quarters to SBUF fp16; DVE tensor_max pairs the odd
# sub-quarter's PSUM against the copy (one PSUM operand - HW rule) writing
# the root segment directly. The first GS columns of one odd sub-quarter
# are ACT-copied instead and self-maxed on DVE at fp16 2x to balance
# ACT/DVE load.
SQ = 1024                 # sub-quarter width
GS = 352                  # self-pair slice width (on sub-quarter 5)


def build_core_kernel(nc, mb=B // P, gs=GS, gs_map=None):
    # gs_map: optional {odd sub-quarter index -> self-slice width}
    if gs_map is None:
        gs_map = {5: gs}
    b = mb * P
    nsq = KC // SQ            # 8 sub-quarters

    t8 = nc.dram_tensor("t8", [2 * P, b], F8, kind="ExternalInput")
    w8 = nc.dram_tensor("w8", [2 * P, KC], F8, kind="ExternalInput")
    root_d = nc.dram_tensor("roots", [b, NG], F16, kind="ExternalOutput")

    with tile.TileContext(nc) as tc:
        with (
            tc.tile_pool(name="persist", bufs=1) as persist,
            tc.tile_pool(name="s", bufs=3) as sp,
            tc.tile_pool(name="s2", bufs=3) as s2p,
            tc.tile_pool(name="root", bufs=3) as rootp,
            tc.tile_pool(name="psum", bufs=4, space="PSUM") as psump,
        ):
            Tn = persist.tile([P, 2 * b], F8)
            Wn = persist.tile([P, 2 * KC], F8)
            # first T chunk (block 0 needs it) via HWDGE; W via gpsimd SWDGE
            # (separate descriptor generator) interleaved by half so early
            # matmuls unblock ASAP (subtile deps)
            ldb = min(512, b)
            for h in range(2):
                nc.sync.dma_start(
                    out=Tn[:, h * b : h * b + ldb], in_=t8[h * P : (h + 1) * P, 0:ldb]
                )
            ldk = 1024
            for c0 in range(0, KC, ldk):
                for h in range(2):
                    # halves go through separate descriptor generators
                    # (HWDGE via sync, SWDGE via gpsimd) to overlap dge time
                    eng_dma = nc.sync if h == 0 else nc.gpsimd
                    eng_dma.dma_start(
                        out=Wn[:, h * KC + c0 : h * KC + c0 + ldk],
                        in_=w8[h * P : (h + 1) * P, c0 : c0 + ldk],
                    )
            for c0 in range(ldb, b, 2048):
                ld = min(2048, b - c0)
                for h in range(2):
                    eng_dma = nc.sync if h == 0 else nc.gpsimd
                    eng_dma.dma_start(
                        out=Tn[:, h * b + c0 : h * b + c0 + ld],
                        in_=t8[h * P : (h + 1) * P, c0 : c0 + ld],
                    )
            Tv = Tn[:].rearrange("p (two b) -> p two b", two=2)
            Wv = Wn[:].rearrange("p (two n) -> p two n", two=2)

            for m in range(mb):
                lhsT = Tv[:, :, m * P : (m + 1) * P]
                root = rootp.tile([P, NG], F16)
                Seven = None
                for sq in range(nsq):
                    pq = psump.tile([P, SQ], F32, space="PSUM")
                    for i in range(SQ // 512):
                        k0 = sq * SQ + i * 512
                        nc.tensor.matmul(
                            out=pq[:, i * 512 : (i + 1) * 512],
                            lhsT=lhsT,
                            rhs=Wv[:, :, k0 : k0 + 512],
                            start=True,
                            stop=True,
                            perf_mode=DR,
                        )
                    h = sq // 2
                    rseg = root[:, h * SQ : (h + 1) * SQ]
                    if sq % 2 == 0:
                        # ACT: copy the whole even sub-quarter
                        Seven = sp.tile([P, SQ], F16)
                        nc.scalar.activation(Seven[:], pq[:], AF.Copy, bias=0.0)
                    else:
                        g0 = gs_map.get(sq, 0)
                        if g0:
                            # ACT: copy the self-slice of the odd sub-quarter
                            Sodd = s2p.tile([P, g0], F16)
                            nc.scalar.activation(
                                Sodd[:], pq[:, 0:g0], AF.Copy, bias=0.0
                            )
                            # DVE fp16 2x self-max
                            nc.vector.tensor_max(
                                rseg[:, 0:g0], Sodd[:], Seven[:, 0:g0]
                            )
                        # DVE: root = max(odd PSUM, even copy)
                        nc.vector.tensor_max(
                            rseg[:, g0:SQ], pq[:, g0:SQ], Seven[:, g0:SQ]
                        )
                # per-h root DMAs release the root tile progressively and
                # shrink the end-of-block drain tail
                for hh in range(4):
                    nc.sync.dma_start(
                        out=root_d[m * P : (m + 1) * P, hh * SQ : (hh + 1) * SQ],
                        in_=root[:, hh * SQ : (hh + 1) * SQ],
                    )

    nc.compile()
    return nc


_CACHE = {}
LAST_RESULT = None
LAST_AMB = -1


def _get_nc():
    if "nc" not in _CACHE:
        nc = bacc.Bacc(
            "TRN2", target_bir_lowering=False, debug=False, enable_asserts=False
        )
        build_core_kernel(nc)
        _CACHE["nc"] = nc
    return _CACHE["nc"]


def kernel(targ: np.ndarray, W: np.ndarray) -> np.ndarray:
    assert targ.shape == (B, D) and W.shape == (D, K)
    targ = np.ascontiguousarray(targ, dtype=np.float32)
    W = np.ascontiguousarray(W, dtype=np.float32)
    nc = _get_nc()

    # host prep: normalize, scale, fp8e4-quantize
    rown = np.linalg.norm(targ, axis=1)
    t8 = np.ascontiguousarray(
        (SCALE * (targ / rown[:, None])).T.astype(ml_dtypes.float8_e4m3)
    )  # [256, B]
    coln = np.linalg.norm(W, axis=0)
    W8 = (SCALE * (W / coln[None, :])).astype(ml_dtypes.float8_e4m3)  # [256, K]

    in_maps = []
    for c in range(NCORES):
        in_maps.append(
            {"t8": t8, "w8": np.ascontiguousarray(W8[:, c * KC : (c + 1) * KC])}
        )

    global LAST_RESULT, LAST_AMB
    LAST_RESULT = run_bass_kernel_spmd(nc, in_maps, list(range(NCORES)))
    res = LAST_RESULT.results

    # roots: [NCORES, B, NG] -> [B, NCORES*NG]; pair gid = core*NG + local
    roots = np.concatenate(
        [r["roots"].astype(np.float32) for r in res], axis=1
    )  # [B, 32768]

    # top-T pair groups + (T+1)-th root for the bound
    part = np.argpartition(-roots, T_GROUPS, axis=1)
    topT = part[:, :T_GROUPS]  # [B, T]
    rT1 = np.take_along_axis(roots, part[:, T_GROUPS : T_GROUPS + 1], axis=1)[:, 0]

    # decode pair gid -> 2 candidate code columns: local gid j = h*SQ + n
    # pairs codes {h*2*SQ + n, h*2*SQ + SQ + n} of that core's slab
    core_id = topT // NG
    within = topT % NG
    h = within // SQ
    n = within % SQ
    k1 = core_id * KC + h * 2 * SQ + n  # [B, T]
    cands = np.stack([k1, k1 + SQ], axis=2).reshape(B, T_GROUPS * C)

    # exact rescore (f64) in row chunks
    Wt = np.ascontiguousarray(W.T)  # [K, D]
    best_cos = np.empty(B, dtype=np.float64)
    second_cos = np.empty(B, dtype=np.float64)
    best_k = np.empty(B, dtype=np.int64)
    CH = 1024
    for r0 in range(0, B, CH):
        r1 = min(B, r0 + CH)
        vec = Wt[cands[r0:r1]].astype(np.float64)  # [ch, 2T, D]
        tch = targ[r0:r1].astype(np.float64)
        dots = np.einsum("rcd,rd->rc", vec, tch)
        cn = np.linalg.norm(vec, axis=2)
        cos = dots / (rown[r0:r1, None].astype(np.float64) * cn + EPS)
        o = np.argsort(-cos, axis=1)
        nr = r1 - r0
        best = o[:, 0]
        best_cos[r0:r1] = cos[np.arange(nr), best]
        second_cos[r0:r1] = cos[np.arange(nr), o[:, 1]]
        best_k[r0:r1] = cands[r0:r1][np.arange(nr), best]

    out = Wt[best_k].astype(np.float32)  # [B, D]

    # bound: any code outside the top-T pairs has screen <= rT1 (+ulp),
    # so exact cos <= (rT1+ulp)/256 + BAND. Flag rows where the best
    # candidate doesn't clearly beat that, plus near-ties among candidates.
    bound = (rT1.astype(np.float64) + ULP16) / (SCALE * SCALE) + BAND
    amb = np.where((best_cos < bound) | (best_cos - second_cos < 1e-6))[0]
    LAST_AMB = len(amb)
    if len(amb):
        # full fp32 mirror of the reference for flagged rows
        s = (targ[amb] @ W) / (rown[amb][:, None] * coln[None, :] + EPS)
        k_star = np.argmax(s, axis=1)
        out[amb] = W[:, k_star].T
    return out


if __name__ == "__main__":
    # micro-test: mb blocks, 1 core, random data -> roots vs numpy + sim time
    import time

    from concourse.timeline_sim import TimelineSim

    nc2 = None
    for mb in (2, 6):
        nc = bacc.Bacc(
            "TRN2", target_bir_lowering=False, debug=False, enable_asserts=False
        )
        build_core_kernel(nc, mb=mb)
        t0 = time.time()
        dur = TimelineSim(nc, trace=False).simulate()
        print(
            f"TimelineSim mb={mb}: {dur:.0f} ns (per-block est "
            f"{(dur - 29583 if mb == 6 else 0)/4 if mb == 6 else 0:.0f})",
            flush=True,
        )
        if mb == 2:
            nc2 = nc

    rng = np.random.default_rng(1)
    mb = 2
    t8 = (rng.standard_normal((256, mb * P))).astype(ml_dtypes.float8_e4m3)
    w8 = (rng.standard_normal((256, KC)) * 0.5).astype(ml_dtypes.float8_e4m3)
    res = run_bass_kernel_spmd(nc2, [{"t8": t8, "w8": w8}], [0])
    roots = res.results[0]["roots"].astype(np.float32)  # [mb*P, 4096]
    screen = t8.astype(np.float32).T @ w8.astype(np.float32)
    ref_roots = (
        screen.reshape(mb * P, 4, 2, SQ)
        .max(axis=2)
        .reshape(mb * P, NG)
        .astype(np.float16)
        .astype(np.float32)
    )
    err = np.abs(roots - ref_roots)
    print(
        f"roots err: max {err.max():.5f} rel {err.max()/np.abs(ref_roots).max():.2e}",
        flush=True,
    )


# revision 8
# speedup vs baseline: 1.0254x; 1.0042x over previous
"""vq_codebook trn2 kernel v2: fp8e4 DoubleRow screen + pair-max funnel.

Per core (K sharded 8 ways, slab Kc=8192):
  - host pre-normalizes W columns / t rows, scales by 16, quantizes to fp8e4
  - PE: DoubleRow fp8 matmuls (full d=256 contraction per op) -> PSUM fp32
    screen values = 256 * (cos + quantization err), |err| <= BAND
  - funnel: codes j and j+1024 (adjacent 1024-wide PSUM sub-quarters) form a
    pair-group. ACT copies even sub-quarters to SBUF fp16; one DVE tensor_max
    (single PSUM operand - HW rule) drains the odd sub-quarter and writes the
    pair-max root directly. A tunable slice (GS cols of one odd sub-quarter)
    is ACT-copied too and self-maxed on DVE at fp16 2x to balance ACT vs DVE
    load. No tree, no on-device argmax.
  - roots [128, 4096] fp16 per row-block DMA'd to HBM
Host: top-T pair-groups per row over the 32768 roots, exact rescore of the
2T candidate codes (f64), bound check vs (T+1)-th root + BAND; flagged rows
get a full fp32 row rescore mirroring the reference.
"""

import os
import sys

import numpy as np
import ml_dtypes

for _p in ("/opt/trn_rl_repo", "/root/.axon_site/_ro/trn_rl_repo"):
    if os.path.isdir(_p) and _p not in sys.path:
        sys.path.append(_p)

import concourse.bass as bass
import concourse.tile as tile
from concourse import bacc, mybir
from concourse.bass_utils import run_bass_kernel_spmd

P = 128
B, D, K, NCORES = 8192, 256, 65536, 8
KC = K // NCORES          # 8192 codes per core
C = 2                     # codes per root group (adjacent pair)
QW = 2048                 # PSUM quarter width
GQ = QW // C              # pair-groups per quarter (1024)
NG = KC // C              # 4096 roots per core row
SCALE = 16.0
EPS = 1e-7
BAND = 0.0155             # |screen/256 - cos| bound (measured 0.0133 + margin)
ULP16 = 0.1               # fp16 ulp at screen magnitude ~90 (raw units)
T_GROUPS = 64             # pair-groups rescored exactly per row

F32 = mybir.dt.float32
F16 = mybir.dt.float16
F8 = mybir.dt.float8e4
AF = mybir.ActivationFunctionType
DR = mybir.MatmulPerfMode.DoubleRow

# Codes j and j+SQ (adjacent 1024-wide sub-quarters) form a pair-group:
# ACT copies even sub-quarters to SBUF fp16; DVE tensor_max pairs the odd
# sub-quarter's PSUM against the copy (one PSUM operand - HW rule) writing
# the root segment directly. The first GS columns of one odd sub-quarter
# are ACT-copied instead and self-maxed on DVE at fp16 2x to balance
# ACT/DVE load.
SQ = 1024                 # sub-quarter width
GS = 352                  # self-pair slice width (on sub-quarter 5)


def build_core_kernel(nc, mb=B // P, gs=GS, gs_map=None):
    # gs_map: optional {odd sub-quarter index -> self-slice width}
    if gs_map is None:
        gs_map = {5: gs}
    b = mb * P
    nsq = KC // SQ            # 8 sub-quarters

    t8 = nc.dram_tensor("t8", [2 * P, b], F8, kind="ExternalInput")
    w8 = nc.dram_tensor("w8", [2 * P, KC], F8, kind="ExternalInput")
    root_d = nc.dram_tensor("roots", [b, NG], F16, kind="ExternalOutput")

    with tile.TileContext(nc) as tc:
        with (
            tc.tile_pool(name="persist", bufs=1) as persist,
            tc.tile_pool(name="s", bufs=3) as sp,
            tc.tile_pool(name="s2", bufs=3) as s2p,
            tc.tile_pool(name="root", bufs=3) as rootp,
            tc.tile_pool(name="psum", bufs=4, space="PSUM") as psump,
        ):
            Tn = persist.tile([P, 2 * b], F8)
            Wn = persist.tile([P, 2 * KC], F8)
            # first T chunk (block 0 needs it) via HWDGE; W via gpsimd SWDGE
            # (separate descriptor generator) interleaved by half so early
            # matmuls unblock ASAP (subtile deps)
            ldb = min(512, b)
            for h in range(2):
                nc.sync.dma_start(
                    out=Tn[:, h * b : h * b + ldb], in_=t8[h * P : (h + 1) * P, 0:ldb]
                )
            ldk = 1024
            for c0 in range(0, KC, ldk):
                for h in range(2):
                    # halves go through separate descriptor generators
                    # (HWDGE via sync, SWDGE via gpsimd) to overlap dge time;
                    # the very first pair stays on the lower-latency HWDGE
                    # path so block 0's matmuls unblock sooner
                    eng_dma = nc.sync if (h == 0 or c0 == 0) else nc.gpsimd
                    eng_dma.dma_start(
                        out=Wn[:, h * KC + c0 : h * KC + c0 + ldk],
                        in_=w8[h * P : (h + 1) * P, c0 : c0 + ldk],
                    )
            for c0 in range(ldb, b, 2048):
                ld = min(2048, b - c0)
                for h in range(2):
                    eng_dma = nc.sync if h == 0 else nc.gpsimd
                    eng_dma.dma_start(
                        out=Tn[:, h * b + c0 : h * b + c0 + ld],
                        in_=t8[h * P : (h + 1) * P, c0 : c0 + ld],
                    )
            Tv = Tn[:].rearrange("p (two b) -> p two b", two=2)
            Wv = Wn[:].rearrange("p (two n) -> p two n", two=2)

            for m in range(mb):
                lhsT = Tv[:, :, m * P : (m + 1) * P]
                root = rootp.tile([P, NG], F16)
                Seven = None
                for sq in range(nsq):
                    pq = psump.tile([P, SQ], F32, space="PSUM")
                    for i in range(SQ // 512):
                        k0 = sq * SQ + i * 512
                        nc.tensor.matmul(
                            out=pq[:, i * 512 : (i + 1) * 512],
                            lhsT=lhsT,
                            rhs=Wv[:, :, k0 : k0 + 512],
                            start=True,
                            stop=True,
                            perf_mode=DR,
                        )
                    h = sq // 2
                    rseg = root[:, h * SQ : (h + 1) * SQ]
                    if sq % 2 == 0:
                        # ACT: copy the whole even sub-quarter
                        Seven = sp.tile([P, SQ], F16)
                        nc.scalar.activation(Seven[:], pq[:], AF.Copy, bias=0.0)
                    else:
                        g0 = gs_map.get(sq, 0)
                        if g0:
                            # ACT: copy the self-slice of the odd sub-quarter
                            Sodd = s2p.tile([P, g0], F16)
                            nc.scalar.activation(
                                Sodd[:], pq[:, 0:g0], AF.Copy, bias=0.0
                            )
                            # DVE fp16 2x self-max
                            nc.vector.tensor_max(
                                rseg[:, 0:g0], Sodd[:], Seven[:, 0:g0]
                            )
                        # DVE: root = max(odd PSUM, even copy)
                        nc.vector.tensor_max(
                            rseg[:, g0:SQ], pq[:, g0:SQ], Seven[:, g0:SQ]
                        )
                # per-h root DMAs release the root tile progressively and
                # shrink the end-of-block drain tail
                for hh in range(4):
                    nc.sync.dma_start(
                        out=root_d[m * P : (m + 1) * P, hh * SQ : (hh + 1) * SQ],
                        in_=root[:, hh * SQ : (hh + 1) * SQ],
                    )

    nc.compile()
    return nc


_CACHE = {}
LAST_RESULT = None
LAST_AMB = -1


def _get_nc():
    if "nc" not in _CACHE:
        nc = bacc.Bacc(
            "TRN2", target_bir_lowering=False, debug=False, enable_asserts=False
        )
        build_core_kernel(nc)
        _CACHE["nc"] = nc
    return _CACHE["nc"]


def kernel(targ: np.ndarray, W: np.ndarray) -> np.ndarray:
    assert targ.shape == (B, D) and W.shape == (D, K)
    targ = np.ascontiguousarray(targ, dtype=np.float32)
    W = np.ascontiguousarray(W, dtype=np.float32)
    nc = _get_nc()

    # host prep: normalize, scale, fp8e4-quantize
    rown = np.linalg.norm(targ, axis=1)
    t8 = np.ascontiguousarray(
        (SCALE * (targ / rown[:, None])).T.astype(ml_dtypes.float8_e4m3)
    )  # [256, B]
    coln = np.linalg.norm(W, axis=0)
    W8 = (SCALE * (W / coln[None, :])).astype(ml_dtypes.float8_e4m3)  # [256, K]

    in_maps = []
    for c in range(NCORES):
        in_maps.append(
            {"t8": t8, "w8": np.ascontiguousarray(W8[:, c * KC : (c + 1) * KC])}
        )

    global LAST_RESULT, LAST_AMB
    LAST_RESULT = run_bass_kernel_spmd(nc, in_maps, list(range(NCORES)))
    res = LAST_RESULT.results

    # roots: [NCORES, B, NG] -> [B, NCORES*NG]; pair gid = core*NG + local
    roots = np.concatenate(
        [r["roots"].astype(np.float32) for r in res], axis=1
    )  # [B, 32768]

    # top-T pair groups + (T+1)-th root for the bound
    part = np.argpartition(-roots, T_GROUPS, axis=1)
    topT = part[:, :T_GROUPS]  # [B, T]
    rT1 = np.take_along_axis(roots, part[:, T_GROUPS : T_GROUPS + 1], axis=1)[:, 0]

    # decode pair gid -> 2 candidate code columns: local gid j = h*SQ + n
    # pairs codes {h*2*SQ + n, h*2*SQ + SQ + n} of that core's slab
    core_id = topT // NG
    within = topT % NG
    h = within // SQ
    n = within % SQ
    k1 = core_id * KC + h * 2 * SQ + n  # [B, T]
    cands = np.stack([k1, k1 + SQ], axis=2).reshape(B, T_GROUPS * C)

    # exact rescore (f64) in row chunks
    Wt = np.ascontiguousarray(W.T)  # [K, D]
    best_cos = np.empty(B, dtype=np.float64)
    second_cos = np.empty(B, dtype=np.float64)
    best_k = np.empty(B, dtype=np.int64)
    CH = 1024
    for r0 in range(0, B, CH):
        r1 = min(B, r0 + CH)
        vec = Wt[cands[r0:r1]].astype(np.float64)  # [ch, 2T, D]
        tch = targ[r0:r1].astype(np.float64)
        dots = np.einsum("rcd,rd->rc", vec, tch)
        cn = np.linalg.norm(vec, axis=2)
        cos = dots / (rown[r0:r1, None].astype(np.float64) * cn + EPS)
        o = np.argsort(-cos, axis=1)
        nr = r1 - r0
        best = o[:, 0]
        best_cos[r0:r1] = cos[np.arange(nr), best]
        second_cos[r0:r1] = cos[np.arange(nr), o[:, 1]]
        best_k[r0:r1] = cands[r0:r1][np.arange(nr), best]

    out = Wt[best_k].astype(np.float32)  # [B, D]

    # bound: any code outside the top-T pairs has screen <= rT1 (+ulp),
    # so exact cos <= (rT1+ulp)/256 + BAND. Flag rows where the best
    # candidate doesn't clearly beat that, plus near-ties among candidates.
    bound = (rT1.astype(np.float64) + ULP16) / (SCALE * SCALE) + BAND
    amb = np.where((best_cos < bound) | (best_cos - second_cos < 1e-6))[0]
    LAST_AMB = len(amb)
    if len(amb):
        # full fp32 mirror of the reference for flagged rows
        s = (targ[amb] @ W) / (rown[amb][:, None] * coln[None, :] + EPS)
        k_star = np.argmax(s, axis=1)
        out[amb] = W[:, k_star].T
    return out


if __name__ == "__main__":
    # micro-test: mb blocks, 1 core, random data -> roots vs numpy + sim time
    import time

    from concourse.timeline_sim import TimelineSim

    nc2 = None
    for mb in (2, 6):
        nc = bacc.Bacc(
            "TRN2", target_bir_lowering=False, debug=False, enable_asserts=False
        )
        build_core_kernel(nc, mb=mb)
        t0 = time.time()
        dur = TimelineSim(nc, trace=False).simulate()
        print(
            f"TimelineSim mb={mb}: {dur:.0f} ns (per-block est "
            f"{(dur - 29583 if mb == 6 else 0)/4 if mb == 6 else 0:.0f})",
            flush=True,
        )
        if mb == 2:
            nc2 = nc

    rng = np.random.default_rng(1)
    mb = 2
    t8 = (rng.standard_normal((256, mb * P))).astype(ml_dtypes.float8_e4m3)
    w8 = (rng.standard_normal((256, KC)) * 0.5).astype(ml_dtypes.float8_e4m3)
    res = run_bass_kernel_spmd(nc2, [{"t8": t8, "w8": w8}], [0])
    roots = res.results[0]["roots"].astype(np.float32)  # [mb*P, 4096]
    screen = t8.astype(np.float32).T @ w8.astype(np.float32)
    ref_roots = (
        screen.reshape(mb * P, 4, 2, SQ)
        .max(axis=2)
        .reshape(mb * P, NG)
        .astype(np.float16)
        .astype(np.float32)
    )
    err = np.abs(roots - ref_roots)
    print(
        f"roots err: max {err.max():.5f} rel {err.max()/np.abs(ref_roots).max():.2e}",
        flush=True,
    )


# revision 9
# speedup vs baseline: 1.0263x; 1.0008x over previous
"""vq_codebook trn2 kernel v2: fp8e4 DoubleRow screen + pair-max funnel.

Per core (K sharded 8 ways, slab Kc=8192):
  - host pre-normalizes W columns / t rows, scales by 16, quantizes to fp8e4
  - PE: DoubleRow fp8 matmuls (full d=256 contraction per op) -> PSUM fp32
    screen values = 256 * (cos + quantization err), |err| <= BAND
  - funnel: codes j and j+1024 (adjacent 1024-wide PSUM sub-quarters) form a
    pair-group. ACT copies even sub-quarters to SBUF fp16; one DVE tensor_max
    (single PSUM operand - HW rule) drains the odd sub-quarter and writes the
    pair-max root directly. A tunable slice (GS cols of one odd sub-quarter)
    is ACT-copied too and self-maxed on DVE at fp16 2x to balance ACT vs DVE
    load. No tree, no on-device argmax.
  - roots [128, 4096] fp16 per row-block DMA'd to HBM
Host: top-T pair-groups per row over the 32768 roots, exact rescore of the
2T candidate codes (f64), bound check vs (T+1)-th root + BAND; flagged rows
get a full fp32 row rescore mirroring the reference.
"""

import os
import sys

import numpy as np
import ml_dtypes

for _p in ("/opt/trn_rl_repo", "/root/.axon_site/_ro/trn_rl_repo"):
    if os.path.isdir(_p) and _p not in sys.path:
        sys.path.append(_p)

import concourse.bass as bass
import concourse.tile as tile
from concourse import bacc, mybir
from concourse.bass_utils import run_bass_kernel_spmd

P = 128
B, D, K, NCORES = 8192, 256, 65536, 8
KC = K // NCORES          # 8192 codes per core
C = 2                     # codes per root group (adjacent pair)
QW = 2048                 # PSUM quarter width
GQ = QW // C              # pair-groups per quarter (1024)
NG = KC // C              # 4096 roots per core row
SCALE = 16.0
EPS = 1e-7
BAND = 0.0155             # |screen/256 - cos| bound (measured 0.0133 + margin)
ULP16 = 0.1               # fp16 ulp at screen magnitude ~90 (raw units)
T_GROUPS = 64             # pair-groups rescored exactly per row

F32 = mybir.dt.float32
F16 = mybir.dt.float16
F8 = mybir.dt.float8e4
AF = mybir.ActivationFunctionType
DR = mybir.MatmulPerfMode.DoubleRow

# Codes j and j+SQ (adjacent 1024-wide sub-quarters) form a pair-group:
# ACT copies even sub-quarters to SBUF fp16; DVE tensor_max pairs the odd
# sub-quarter's PSUM against the copy (one PSUM operand - HW rule) writing
# the root segment directly. The first GS columns of one odd sub-quarter
# are ACT-copied instead and self-maxed on DVE at fp16 2x to balance
# ACT/DVE load.
SQ = 1024                 # sub-quarter width
GS = 360                  # self-pair slice width (on sub-quarter 5)


def build_core_kernel(nc, mb=B // P, gs=GS, gs_map=None):
    # gs_map: optional {odd sub-quarter index -> self-slice width}
    if gs_map is None:
        gs_map = {5: gs}
    b = mb * P
    nsq = KC // SQ            # 8 sub-quarters

    t8 = nc.dram_tensor("t8", [2 * P, b], F8, kind="ExternalInput")
    w8 = nc.dram_tensor("w8", [2 * P, KC], F8, kind="ExternalInput")
    root_d = nc.dram_tensor("roots", [b, NG], F16, kind="ExternalOutput")

    with tile.TileContext(nc) as tc:
        with (
            tc.tile_pool(name="persist", bufs=1) as persist,
            tc.tile_pool(name="s", bufs=3) as sp,
            tc.tile_pool(name="s2", bufs=3) as s2p,
            tc.tile_pool(name="root", bufs=3) as rootp,
            tc.tile_pool(name="psum", bufs=4, space="PSUM") as psump,
        ):
            Tn = persist.tile([P, 2 * b], F8)
            Wn = persist.tile([P, 2 * KC], F8)
            # first T chunk (block 0 needs it) via HWDGE; W via gpsimd SWDGE
            # (separate descriptor generator) interleaved by half so early
            # matmuls unblock ASAP (subtile deps)
            ldb = min(512, b)
            for h in range(2):
                nc.sync.dma_start(
                    out=Tn[:, h * b : h * b + ldb], in_=t8[h * P : (h + 1) * P, 0:ldb]
                )
            ldk = 1024
            for c0 in range(0, KC, ldk):
                for h in range(2):
                    # halves go through separate descriptor generators
                    # (HWDGE via sync, SWDGE via gpsimd) to overlap dge time;
                    # the very first pair stays on the lower-latency HWDGE
                    # path so block 0's matmuls unblock sooner
                    eng_dma = nc.sync if (h == 0 or c0 == 0) else nc.gpsimd
                    eng_dma.dma_start(
                        out=Wn[:, h * KC + c0 : h * KC + c0 + ldk],
                        in_=w8[h * P : (h + 1) * P, c0 : c0 + ldk],
                    )
            for c0 in range(ldb, b, 2048):
                ld = min(2048, b - c0)
                for h in range(2):
                    eng_dma = nc.sync if h == 0 else nc.gpsimd
                    eng_dma.dma_start(
                        out=Tn[:, h * b + c0 : h * b + c0 + ld],
                        in_=t8[h * P : (h + 1) * P, c0 : c0 + ld],
                    )
            Tv = Tn[:].rearrange("p (two b) -> p two b", two=2)
            Wv = Wn[:].rearrange("p (two n) -> p two n", two=2)

            for m in range(mb):
                lhsT = Tv[:, :, m * P : (m + 1) * P]
                root = rootp.tile([P, NG], F16)
                Seven = None
                for sq in range(nsq):
                    pq = psump.tile([P, SQ], F32, space="PSUM")
                    for i in range(SQ // 512):
                        k0 = sq * SQ + i * 512
                        nc.tensor.matmul(
                            out=pq[:, i * 512 : (i + 1) * 512],
                            lhsT=lhsT,
                            rhs=Wv[:, :, k0 : k0 + 512],
                            start=True,
                            stop=True,
                            perf_mode=DR,
                        )
                    h = sq // 2
                    rseg = root[:, h * SQ : (h + 1) * SQ]
                    if sq % 2 == 0:
                        # ACT: copy the whole even sub-quarter
                        Seven = sp.tile([P, SQ], F16)
                        nc.scalar.activation(Seven[:], pq[:], AF.Copy, bias=0.0)
                    else:
                        g0 = gs_map.get(sq, 0)
                        if g0:
                            # ACT: copy the self-slice of the odd sub-quarter
                            Sodd = s2p.tile([P, g0], F16)
                            nc.scalar.activation(
                                Sodd[:], pq[:, 0:g0], AF.Copy, bias=0.0
                            )
                            # DVE fp16 2x self-max
                            nc.vector.tensor_max(
                                rseg[:, 0:g0], Sodd[:], Seven[:, 0:g0]
                            )
                        # DVE: root = max(odd PSUM, even copy)
                        nc.vector.tensor_max(
                            rseg[:, g0:SQ], pq[:, g0:SQ], Seven[:, g0:SQ]
                        )
                # per-h root DMAs release the root tile progressively and
                # shrink the end-of-block drain tail
                for hh in range(4):
                    nc.sync.dma_start(
                        out=root_d[m * P : (m + 1) * P, hh * SQ : (hh + 1) * SQ],
                        in_=root[:, hh * SQ : (hh + 1) * SQ],
                    )

    nc.compile()
    return nc


_CACHE = {}
LAST_RESULT = None
LAST_AMB = -1


def _get_nc():
    if "nc" not in _CACHE:
        nc = bacc.Bacc(
            "TRN2", target_bir_lowering=False, debug=False, enable_asserts=False
        )
        build_core_kernel(nc)
        _CACHE["nc"] = nc
    return _CACHE["nc"]


def kernel(targ: np.ndarray, W: np.ndarray) -> np.ndarray:
    assert targ.shape == (B, D) and W.shape == (D, K)
    targ = np.ascontiguousarray(targ, dtype=np.float32)
    W = np.ascontiguousarray(W, dtype=np.float32)
    nc = _get_nc()

    # host prep: normalize, scale, fp8e4-quantize
    rown = np.linalg.norm(targ, axis=1)
    t8 = np.ascontiguousarray(
        (SCALE * (targ / rown[:, None])).T.astype(ml_dtypes.float8_e4m3)
    )  # [256, B]
    coln = np.linalg.norm(W, axis=0)
    W8 = (SCALE * (W / coln[None, :])).astype(ml_dtypes.float8_e4m3)  # [256, K]

    in_maps = []
    for c in range(NCORES):
        in_maps.append(
            {"t8": t8, "w8": np.ascontiguousarray(W8[:, c * KC : (c + 1) * KC])}
        )

    global LAST_RESULT, LAST_AMB
    LAST_RESULT = run_bass_kernel_spmd(nc, in_maps, list(range(NCORES)))
    res = LAST_RESULT.results

    # roots: [NCORES, B, NG] -> [B, NCORES*NG]; pair gid = core*NG + local
    roots = np.concatenate(
        [r["roots"].astype(np.float32) for r in res], axis=1
    )  # [B, 32768]

    # top-T pair groups + (T+1)-th root for the bound
    part = np.argpartition(-roots, T_GROUPS, axis=1)
    topT = part[:, :T_GROUPS]  # [B, T]
    rT1 = np.take_along_axis(roots, part[:, T_GROUPS : T_GROUPS + 1], axis=1)[:, 0]

    # decode pair gid -> 2 candidate code columns: local gid j = h*SQ + n
    # pairs codes {h*2*SQ + n, h*2*SQ + SQ + n} of that core's slab
    core_id = topT // NG
    within = topT % NG
    h = within // SQ
    n = within % SQ
    k1 = core_id * KC + h * 2 * SQ + n  # [B, T]
    cands = np.stack([k1, k1 + SQ], axis=2).reshape(B, T_GROUPS * C)

    # exact rescore (f64) in row chunks
    Wt = np.ascontiguousarray(W.T)  # [K, D]
    best_cos = np.empty(B, dtype=np.float64)
    second_cos = np.empty(B, dtype=np.float64)
    best_k = np.empty(B, dtype=np.int64)
    CH = 1024
    for r0 in range(0, B, CH):
        r1 = min(B, r0 + CH)
        vec = Wt[cands[r0:r1]].astype(np.float64)  # [ch, 2T, D]
        tch = targ[r0:r1].astype(np.float64)
        dots = np.einsum("rcd,rd->rc", vec, tch)
        cn = np.linalg.norm(vec, axis=2)
        cos = dots / (rown[r0:r1, None].astype(np.float64) * cn + EPS)
        o = np.argsort(-cos, axis=1)
        nr = r1 - r0
        best = o[:, 0]
        best_cos[r0:r1] = cos[np.arange(nr), best]
        second_cos[r0:r1] = cos[np.arange(nr), o[:, 1]]
        best_k[r0:r1] = cands[r0:r1][np.arange(nr), best]

    out = Wt[best_k].astype(np.float32)  # [B, D]

    # bound: any code outside the top-T pairs has screen <= rT1 (+ulp),
    # so exact cos <= (rT1+ulp)/256 + BAND. Flag rows where the best
    # candidate doesn't clearly beat that, plus near-ties among candidates.
    bound = (rT1.astype(np.float64) + ULP16) / (SCALE * SCALE) + BAND
    amb = np.where((best_cos < bound) | (best_cos - second_cos < 1e-6))[0]
    LAST_AMB = len(amb)
    if len(amb):
        # full fp32 mirror of the reference for flagged rows
        s = (targ[amb] @ W) / (rown[amb][:, None] * coln[None, :] + EPS)
        k_star = np.argmax(s, axis=1)
        out[amb] = W[:, k_star].T
    return out


if __name__ == "__main__":
    # micro-test: mb blocks, 1 core, random data -> roots vs numpy + sim time
    import time

    from concourse.timeline_sim import TimelineSim

    nc2 = None
    for mb in (2, 6):
        nc = bacc.Bacc(
            "TRN2", target_bir_lowering=False, debug=False, enable_asserts=False
        )
        build_core_kernel(nc, mb=mb)
        t0 = time.time()
        dur = TimelineSim(nc, trace=False).simulate()
        print(
            f"TimelineSim mb={mb}: {dur:.0f} ns (per-block est "
            f"{(dur - 29583 if mb == 6 else 0)/4 if mb == 6 else 0:.0f})",
            flush=True,
        )
        if mb == 2:
            nc2 = nc

    rng = np.random.default_rng(1)
    mb = 2
    t8 = (rng.standard_normal((256, mb * P))).astype(ml_dtypes.float8_e4m3)
    w8 = (rng.standard_normal((256, KC)) * 0.5).astype(ml_dtypes.float8_e4m3)
    res = run_bass_kernel_spmd(nc2, [{"t8": t8, "w8": w8}], [0])
    roots = res.results[0]["roots"].astype(np.float32)  # [mb*P, 4096]
    screen = t8.astype(np.float32).T @ w8.astype(np.float32)
    ref_roots = (
        screen.reshape(mb * P, 4, 2, SQ)
        .max(axis=2)
        .reshape(mb * P, NG)
        .astype(np.float16)
        .astype(np.float32)
    )
    err = np.abs(roots - ref_roots)
    print(
        f"roots err: max {err.max():.5f} rel {err.max()/np.abs(ref_roots).max():.2e}",
        flush=True,
    )
